# revision 20
# baseline (speedup 1.0000x reference)
"""Masked multi-head self-attention on 8 Trainium2 NeuronCores.

Sharding: core c handles batch b = c // 2 and head-group g = c % 2
(8 of 16 heads).  Data-parallel over B, tensor-parallel over heads for
qkv_proj (column split) / out_proj (row split).  The [T,T] causal mask
is exploited structurally (tile skipping); the host verifies the mask
is causal and falls back to numpy otherwise.  Host sums the two
head-group partial outputs per batch and adds bout.

v2: software-pipelined schedule.  Input DMA is chunk-major so matmuls
start early; attention blocks run in diagonal (c,p) order with QKV /
out-projection matmuls woven into the softmax-exp gaps so the tensor
engine never idles; the softmax denominator broadcast is a PE outer
product into the rowsum PSUM bank (no DRAM roundtrip); V bias is added
by the vector engine during PSUM evacuation.
"""

from collections import deque

import numpy as np
import ml_dtypes

BF16 = ml_dtypes.bfloat16

B = 4
T = 2048
D = 1024
H = 16
DK = 64
P = 128
NCORES = 8

KT = D // P            # 8   k-tiles over d_model
TTILES = T // P        # 16  tiles over tokens
NCH = 4                # qi chunks of 512
CH = T // NCH          # 512

_CACHE = {}

# diagonal (c, p) block order: attention availability grows smoothly and
# prerequisites (qk pair p, v tiles <= 4c+3) arrive just in time.
BLOCK_ORDER = [
    (0, 0), (1, 0), (0, 1), (2, 0), (1, 1), (0, 2), (3, 0), (2, 1),
    (1, 2), (0, 3), (3, 1), (2, 2), (1, 3), (3, 2), (2, 3), (3, 3),
]


def _build_program():
    import concourse.bass as bass
    import concourse.tile as tile
    from concourse import bacc, mybir
    from contextlib import ExitStack

    f32 = mybir.dt.float32
    bf16 = mybir.dt.bfloat16
    nc = bacc.Bacc("TRN2", target_bir_lowering=False, debug=False,
                   num_devices=NCORES)

    xt_d = nc.declare_dram_parameter("xt", [P, KT * T], bf16, isOutput=False)
    wqk_d = nc.declare_dram_parameter("wqk", [P, 8 * 1024], bf16, isOutput=False)
    wv_d = nc.declare_dram_parameter("wv", [P, KT * 512], bf16, isOutput=False)
    wout_d = nc.declare_dram_parameter("wout", [P, 4 * 1024], bf16, isOutput=False)
    m01_d = nc.declare_dram_parameter("m01", [P, P], bf16, isOutput=False)
    bqk_d = nc.declare_dram_parameter("bqk", [P, 8], f32, isOutput=False)
    bv_d = nc.declare_dram_parameter("bv", [1, 512], bf16, isOutput=False)
    out_d = nc.declare_dram_parameter("out", [T, D], f32, isOutput=True)

    ts = bass.ts
    EXP = mybir.ActivationFunctionType.Exp

    with tile.TileContext(nc) as tc, ExitStack() as top:
        const = top.enter_context(tc.tile_pool(name="const", bufs=1))
        qk_pool = top.enter_context(tc.tile_pool(name="qk", bufs=1))
        v_pool = top.enter_context(tc.tile_pool(name="v", bufs=1))
        xt_pool = top.enter_context(tc.tile_pool(name="xt", bufs=1))
        wqk_pool = top.enter_context(tc.tile_pool(name="wqk", bufs=2))
        wv_pool = top.enter_context(tc.tile_pool(name="wv", bufs=1))
        at_pool = top.enter_context(tc.tile_pool(name="at", bufs=1))
        wout_pool = top.enter_context(tc.tile_pool(name="wout", bufs=1))
        m01_pool = top.enter_context(tc.tile_pool(name="m01", bufs=1))
        pt_pool = top.enter_context(tc.tile_pool(name="pt", bufs=8))
        rs_pool = top.enter_context(tc.tile_pool(name="rs", bufs=2))
        bc_pool = top.enter_context(tc.tile_pool(name="bc", bufs=2))
        scr_pool = top.enter_context(
            tc.tile_pool(name="scr", bufs=2, space="DRAM"))
        stg_pool = top.enter_context(tc.tile_pool(name="stg", bufs=2))
        osb_pool = top.enter_context(tc.tile_pool(name="osb", bufs=4))
        ps_q = top.enter_context(tc.tile_pool(name="ps_q", bufs=2, space="PSUM"))
        ps_s = top.enter_context(tc.tile_pool(name="ps_s", bufs=2, space="PSUM"))
        ps_at = top.enter_context(tc.tile_pool(name="ps_at", bufs=1, space="PSUM"))
        ps_rs = top.enter_context(tc.tile_pool(name="ps_rs", bufs=1, space="PSUM"))

        ones_col = const.tile([P, 1], bf16, tag="ones_col")
        neg12 = const.tile([P, 1], f32, tag="neg12")
        bqk_sb = const.tile([P, 8], f32, tag="bqk")
        bv_bc = const.tile([P, 512], bf16, tag="bv_bc")
        nc.vector.memset(ones_col[:], 1.0)
        nc.vector.memset(neg12[:], -12.0)
        nc.sync.dma_start(bqk_sb[:], bqk_d[:])
        nc.sync.dma_start(bv_bc[:], bv_d[0:1, :].to_broadcast((P, 512)))
        m01_blk = m01_pool.tile([P, P], bf16, tag="m01")
        nc.sync.dma_start(m01_blk[:], m01_d[:])

        # qkT [1024, T] as 8 tiles (i<4: q of head pair i, pre-scaled 1/8;
        # i>=4: k of pair i-4); v [T, 512] as 16 tiles; attnT as 4 tiles.
        qk = [qk_pool.tile([P, T], bf16, tag=f"qk{i}", name=f"qk{i}")
              for i in range(8)]
        v = [v_pool.tile([P, 512], bf16, tag=f"v{t}", name=f"v{t}")
             for t in range(TTILES)]
        at = [at_pool.tile([P, T], bf16, tag=f"at{p}", name=f"at{p}")
              for p in range(4)]
        xt = [xt_pool.tile([P, T], bf16, tag=f"xt{kt}", name=f"xt{kt}")
              for kt in range(KT)]
        wv_sb = wv_pool.tile([P, KT * 512], bf16, tag="wv")
        wout_sb = wout_pool.tile([P, 4 * 1024], bf16, tag="wout")

        # weight DMAs for pair 0 first, then x chunk-major so the first
        # qk chunk can start after ~1.5 MB of input traffic.
        wqk_sb = {}
        for i in (0, 4):
            wqk_sb[i] = wqk_pool.tile([P, 1024], bf16, tag=f"wqk{i % 2}",
                                      name=f"wqk{i}")
            nc.sync.dma_start(wqk_sb[i][:], wqk_d[:, ts(i, 1024)])
        for kt in range(KT):
            nc.sync.dma_start(xt[kt][:, ts(0, CH)],
                              xt_d[:, kt * T + 0 * CH:kt * T + 1 * CH])
        nc.sync.dma_start(wv_sb[:], wv_d[:])
        for n in range(1, NCH):
            for kt in range(KT):
                nc.sync.dma_start(xt[kt][:, ts(n, CH)],
                                  xt_d[:, kt * T + n * CH:kt * T + (n + 1) * CH])
        nc.sync.dma_start(wout_sb[:], wout_d[:])

        def emit_qk_chunk(i, n):
            """qkT[i][:, chunk n] = (Wqk col-tile i).T @ x.T chunk"""
            acc = ps_q.tile([P, CH], f32, tag="q", name="qkacc")
            for kt in range(KT):
                nc.tensor.matmul(
                    acc[:], wqk_sb[i][:, ts(kt, P)], xt[kt][:, ts(n, CH)],
                    start=(kt == 0), stop=(kt == KT - 1))
            nc.vector.tensor_scalar_add(
                qk[i][:, ts(n, CH)], acc[:], bqk_sb[:, i:i + 1])

        def emit_v(t):
            """v[t] = x-tile.T @ Wv + bv          -> [128 tok, 512 dcol]"""
            acc = ps_q.tile([P, 512], f32, tag="q", name="vacc")
            for kt in range(KT):
                nc.tensor.matmul(
                    acc[:], xt[kt][:, ts(t, P)], wv_sb[:, ts(kt, 512)],
                    start=(kt == 0), stop=(kt == KT - 1))
            nc.vector.tensor_add(v[t][:], acc[:], bv_bc[:])

        def emit_op(c, t, dc):
            """out-projection chain for token tile t, d_model half dc"""
            acc = ps_q.tile([P, 512], f32, tag="q", name="oacc")
            for kk in range(4):
                nc.tensor.matmul(
                    acc[:], at[kk][:, ts(t, P)],
                    wout_sb[:, kk * 1024 + dc * 512:kk * 1024 + dc * 512 + 512],
                    start=(kk == 0), stop=(kk == 3))
            o_sb = osb_pool.tile([P, 512], f32, tag="o_sb")
            nc.vector.tensor_copy(o_sb[:], acc[:])
            nc.sync.dma_start(out_d[ts(t, P), ts(dc, 512)], o_sb[:])

        # ---- filler machinery -------------------------------------------
        # (kind, key, cost_ns, closure).  Fillers are emitted with natural
        # (low-preference) priority; attention blocks are emitted inside
        # tc.high_priority() so the scheduler runs them the moment they are
        # ready and uses fillers only for true PE gaps.
        fillers = deque()

        FILLER_DEMOTE = -10_000_000  # priority += 10M -> lowest preference

        def drain(pred):
            keep = deque()
            with tc.high_priority(offset=FILLER_DEMOTE):
                while fillers:
                    item = fillers.popleft()
                    if pred(item):
                        item[3]()
                    else:
                        keep.append(item)
            fillers.extend(keep)

        def att_block(c, p):
            if c == 0 and p >= 2:
                # pair p reuses pair p-2's weight buffers (same pool tag):
                # all of pair p-2 must be emitted before its wqk load, or
                # the buffer-rotation dependency cycles with queue order.
                drain(lambda it: it[0] == "qk" and it[1][0] == p - 2)
            # prerequisites: qk pair p chunks <= c (k columns up to
            # 512*(c+1), q chunk c), v tiles 0..4c+3
            drain(lambda it: (it[0] == "qk" and it[1][0] == p
                              and it[1][1] <= c)
                  or (it[0] == "v" and it[1] <= 4 * c + 3))
            _att_block_body(c, p)

        def _att_block_body(c, p):
            # Three priority classes: the scores->exp->av chain runs at
            # high priority (keeps tile-position pairs adjacent =>
            # concurrent in the array), rowsums at natural priority (they
            # only gate the block-end recip, not the exp pipeline), and
            # fillers demoted to lowest preference.
            kq = qk[4 + p]
            qq = qk[p]
            nki = 4 * (c + 1)
            attn_ps = ps_at.tile([P, CH], f32, tag="at")
            rs_ps = ps_rs.tile([P, CH], f32, tag="rs")
            for j in range(nki):
                st = (j == 0)
                sp = (j == nki - 1)
                # columns < off of this [ki, qi] tile are fully masked
                off = max(0, P * (j - 4 * c))
                pt = pt_pool.tile([P, 1024], bf16, tag="pt")
                with tc.high_priority():
                    s_ps = ps_s.tile([P, 1024], f32, tag="s", name="s_ps")
                    nc.tensor.matmul(
                        s_ps[:, off:CH], kq[0:DK, ts(j, P)],
                        qq[0:DK, c * CH + off:(c + 1) * CH],
                        start=True, stop=True)
                    nc.tensor.matmul(
                        s_ps[:, CH + off:1024], kq[DK:P, ts(j, P)],
                        qq[DK:P, c * CH + off:(c + 1) * CH],
                        start=True, stop=True)
                    if off > 0:
                        for base in (0, CH):
                            nc.scalar.activation(
                                pt[:, base + off:base + CH],
                                s_ps[:, base + off:base + CH],
                                EXP, bias=neg12[:], scale=1.0)
                    else:
                        nc.scalar.activation(
                            pt[:], s_ps[:], EXP, bias=neg12[:], scale=1.0)
                    if j >= 4 * c:  # tile containing the diagonal block
                        for base in (0, CH):
                            nc.vector.tensor_mul(
                                pt[:, base + off:base + off + P],
                                pt[:, base + off:base + off + P],
                                m01_blk[:])
                    nc.tensor.matmul(
                        attn_ps[0:DK, off:CH], v[j][:, ts(2 * p, DK)],
                        pt[:, off:CH],
                        start=st, stop=sp, skip_group_check=True)
                    nc.tensor.matmul(
                        attn_ps[DK:P, off:CH], v[j][:, ts(2 * p + 1, DK)],
                        pt[:, CH + off:1024],
                        start=st, stop=sp, skip_group_check=True)
                # rowsums gate only the block-end recip: natural priority
                nc.tensor.matmul(
                    rs_ps[0:1, off:CH], ones_col[:], pt[:, off:CH],
                    start=st, stop=sp, skip_group_check=True)
                nc.tensor.matmul(
                    rs_ps[32:33, off:CH], ones_col[:], pt[:, CH + off:1024],
                    start=st, stop=sp, skip_group_check=True)
            with tc.high_priority():
                # single recip over rows 0..32 (covers both heads' sums;
                # rows 1-31 are don't-care) — sub-partition DVE writes at
                # base>0 mis-track against downstream readers.  Frees the
                # rowsum bank immediately.
                rs_sb = rs_pool.tile([33, CH], f32, tag="rs_sb")
                nc.vector.reciprocal_approx_fast(rs_sb[:], rs_ps[0:33, :])
                # stage raw attention to SBUF now: frees the attn PSUM
                # bank in ~0.6us instead of waiting for the broadcast DMA.
                att_sb = stg_pool.tile([P, CH], bf16, tag="stg")
                nc.vector.tensor_copy(att_sb[:], attn_ps[:])
            # broadcast 1/rowsum across partitions via DRAM roundtrip DMA
            # and normalize off the critical path (gates only out-proj).
            scrA = scr_pool.tile([1, CH], f32, tag="scrA")
            scrB = scr_pool.tile([1, CH], f32, tag="scrB")
            nc.sync.dma_start(scrA[:], rs_sb[0:1, :])
            nc.sync.dma_start(scrB[:], rs_sb[32:33, :])
            bc_sb = bc_pool.tile([P, CH], f32, tag="bc")
            nc.sync.dma_start(bc_sb[0:DK, :],
                              scrA[0:1, :].to_broadcast((DK, CH)))
            nc.sync.dma_start(bc_sb[DK:P, :],
                              scrB[0:1, :].to_broadcast((DK, CH)))
            nc.vector.tensor_mul(
                at[p][:, ts(c, CH)], att_sb[:], bc_sb[:])

        # ---- prefix: pair 0 chunk 0 + v0-3, then pipelined attention -----
        emit_qk_chunk(0, 0)
        emit_qk_chunk(4, 0)
        for t in range(4):
            emit_v(t)

        # queue the rest as fillers (qk keyed by (pair, chunk))
        for n in range(1, NCH):
            fillers.append(("qk", (0, n), 1700.0,
                            lambda n=n: emit_qk_chunk(0, n)))
            fillers.append(("qk", (0, n), 1700.0,
                            lambda n=n: emit_qk_chunk(4, n)))
        for p in range(1, 4):
            def load_wqk(p=p):
                for i in (p, p + 4):
                    wqk_sb[i] = wqk_pool.tile(
                        [P, 1024], bf16, tag=f"wqk{i % 2}", name=f"wqk{i}")
                    nc.sync.dma_start(wqk_sb[i][:], wqk_d[:, ts(i, 1024)])
            fillers.append(("qk", (p, 0), 100.0, load_wqk))
            for n in range(NCH):
                fillers.append(("qk", (p, n), 1700.0,
                                lambda p=p, n=n: emit_qk_chunk(p, n)))
                fillers.append(("qk", (p, n), 1700.0,
                                lambda p=p, n=n: emit_qk_chunk(p + 4, n)))
        for t in range(4, TTILES):
            fillers.append(("v", t, 1700.0, lambda t=t: emit_v(t)))

        done_pairs = {c: 0 for c in range(NCH)}
        for c, p in BLOCK_ORDER:
            att_block(c, p)
            done_pairs[c] += 1
            if done_pairs[c] == 4:  # out-projection for chunk c unlocked
                for t in range(4 * c, 4 * c + 4):
                    for dc in range(2):
                        fillers.append(
                            ("op", c, 900.0,
                             lambda c=c, t=t, dc=dc: emit_op(c, t, dc)))
        # drain remaining fillers (tail out-projections)
        drain(lambda it: True)

    nc.compile()
    return nc


def _get_program():
    if "nc" not in _CACHE:
        _CACHE["nc"] = _build_program()
    return _CACHE["nc"]


def _prep_core_inputs(x, attn_mask, Wqkv, bqkv, Wout):
    """Per-core host-side sharding + DMA-friendly layouts."""
    # partial diagonal block: m01[ki_rel, qi_rel] = 1 iff qi_rel >= ki_rel
    m01 = np.triu(np.ones((P, P), np.float32)).astype(BF16)

    in_maps = []
    for core in range(NCORES):
        b, g = core // 2, core % 2
        xt = np.ascontiguousarray(
            x[b].T.reshape(KT, P, T).transpose(1, 0, 2).reshape(P, KT * T)
        ).astype(BF16)
        wq = Wqkv[:, 512 * g:512 * g + 512] * np.float32(0.125)
        wk = Wqkv[:, 1024 + 512 * g:1024 + 512 * g + 512]
        wqk = np.concatenate([wq, wk], axis=1)  # [1024, 1024]
        wqk = np.ascontiguousarray(
            wqk.reshape(KT, P, 8, P).transpose(1, 2, 0, 3).reshape(P, 8192)
        ).astype(BF16)
        wv = Wqkv[:, 2048 + 512 * g:2048 + 512 * g + 512]
        wv = np.ascontiguousarray(
            wv.reshape(KT, P, 512).transpose(1, 0, 2).reshape(P, KT * 512)
        ).astype(BF16)
        wo = Wout[512 * g:512 * g + 512, :]
        wo = np.ascontiguousarray(
            wo.reshape(4, P, 1024).transpose(1, 0, 2).reshape(P, 4096)
        ).astype(BF16)
        bq = bqkv[512 * g:512 * g + 512] * np.float32(0.125)
        bk = bqkv[1024 + 512 * g:1024 + 512 * g + 512]
        bqk = np.ascontiguousarray(
            np.concatenate([bq, bk]).reshape(8, P).T)
        bv = np.ascontiguousarray(
            bqkv[2048 + 512 * g:2048 + 512 * g + 512].reshape(1, 512)
        ).astype(BF16)
        in_maps.append({"xt": xt, "wqk": wqk, "wv": wv, "wout": wo,
                        "m01": m01, "bqk": bqk, "bv": bv})
    return in_maps


def _mask_is_causal(attn_mask):
    zero = (attn_mask == 0.0)
    if not np.array_equal(zero, np.tril(np.ones((T, T), dtype=bool))):
        return False
    return bool(np.all(attn_mask[~zero] <= np.float32(-50.0)))


def _numpy_fallback(x, attn_mask, Wqkv, bqkv, Wout, bout):
    qkv = x @ Wqkv + bqkv
    qkv = qkv.reshape(B, T, 3, H, DK).transpose(2, 0, 3, 1, 4)
    q, k, vv = qkv[0], qkv[1], qkv[2]
    scores = np.einsum("bhqd,bhkd->bhqk", q, k) / np.float32(np.sqrt(DK))
    scores = scores + attn_mask
    scores -= scores.max(axis=-1, keepdims=True)
    e = np.exp(scores)
    probs = e / e.sum(axis=-1, keepdims=True)
    attn = np.einsum("bhqk,bhkd->bhqd", probs, vv)
    attn = attn.transpose(0, 2, 1, 3).reshape(B, T, D)
    return (attn @ Wout + bout).astype(np.float32)


def _run(inputs, trace=False):
    from concourse.bass_utils import run_bass_kernel_spmd

    x = np.asarray(inputs["x"], dtype=np.float32)
    attn_mask = np.asarray(inputs["attn_mask"], dtype=np.float32)
    Wqkv = np.asarray(inputs["Wqkv"], dtype=np.float32)
    bqkv = np.asarray(inputs["bqkv"], dtype=np.float32)
    Wout = np.asarray(inputs["Wout"], dtype=np.float32)
    bout = np.asarray(inputs["bout"], dtype=np.float32)

    if not _mask_is_causal(attn_mask):
        return _numpy_fallback(x, attn_mask, Wqkv, bqkv, Wout, bout), None

    nc = _get_program()
    in_maps = _prep_core_inputs(x, attn_mask, Wqkv, bqkv, Wout)
    res = run_bass_kernel_spmd(nc, in_maps, list(range(NCORES)), trace=trace)
    out = np.empty((B, T, D), np.float32)
    for b in range(B):
        out[b] = res.results[2 * b]["out"] + res.results[2 * b + 1]["out"] + bout
    return out, res.exec_time_ns


def kernel(**inputs) -> np.ndarray:
    out, _ = _run(inputs, trace=False)
    return out


# revision 22
# speedup vs baseline: 1.0181x; 1.0181x over previous
"""Masked multi-head self-attention on 8 Trainium2 NeuronCores.

Sharding: core c handles batch b = c // 2 and head-group g = c % 2
(8 of 16 heads).  Data-parallel over B, tensor-parallel over heads for
qkv_proj (column split) / out_proj (row split).  The [T,T] causal mask
is exploited structurally (tile skipping); the host verifies the mask
is causal and falls back to numpy otherwise.  Host sums the two
head-group partial outputs per batch and adds bout.

v2: software-pipelined schedule.  Input DMA is chunk-major so matmuls
start early; attention blocks run in diagonal (c,p) order with QKV /
out-projection matmuls woven into the softmax-exp gaps so the tensor
engine never idles; the softmax denominator broadcast is a PE outer
product into the rowsum PSUM bank (no DRAM roundtrip); V bias is added
by the vector engine during PSUM evacuation.
"""

from collections import deque

import numpy as np
import ml_dtypes

BF16 = ml_dtypes.bfloat16

B = 4
T = 2048
D = 1024
H = 16
DK = 64
P = 128
NCORES = 8

KT = D // P            # 8   k-tiles over d_model
TTILES = T // P        # 16  tiles over tokens
NCH = 4                # qi chunks of 512
CH = T // NCH          # 512

_CACHE = {}

# diagonal (c, p) block order: attention availability grows smoothly and
# prerequisites (qk pair p, v tiles <= 4c+3) arrive just in time.
BLOCK_ORDER = [
    (0, 0), (1, 0), (0, 1), (2, 0), (1, 1), (0, 2), (3, 0), (2, 1),
    (1, 2), (0, 3), (3, 1), (2, 2), (1, 3), (3, 2), (2, 3), (3, 3),
]


def _build_program():
    import concourse.bass as bass
    import concourse.tile as tile
    from concourse import bacc, mybir
    from contextlib import ExitStack

    f32 = mybir.dt.float32
    bf16 = mybir.dt.bfloat16
    nc = bacc.Bacc("TRN2", target_bir_lowering=False, debug=False,
                   num_devices=NCORES)

    xt_d = nc.declare_dram_parameter("xt", [P, KT * T], bf16, isOutput=False)
    wqk_d = nc.declare_dram_parameter("wqk", [P, 8 * 1024], bf16, isOutput=False)
    wv_d = nc.declare_dram_parameter("wv", [P, KT * 512], bf16, isOutput=False)
    wout_d = nc.declare_dram_parameter("wout", [P, 4 * 1024], bf16, isOutput=False)
    m01_d = nc.declare_dram_parameter("m01", [P, P], bf16, isOutput=False)
    bqk_d = nc.declare_dram_parameter("bqk", [P, 8], f32, isOutput=False)
    bv_d = nc.declare_dram_parameter("bv", [1, 512], bf16, isOutput=False)
    out_d = nc.declare_dram_parameter("out", [T, D], f32, isOutput=True)

    ts = bass.ts
    EXP = mybir.ActivationFunctionType.Exp

    with tile.TileContext(nc) as tc, ExitStack() as top:
        const = top.enter_context(tc.tile_pool(name="const", bufs=1))
        qk_pool = top.enter_context(tc.tile_pool(name="qk", bufs=1))
        v_pool = top.enter_context(tc.tile_pool(name="v", bufs=1))
        xt_pool = top.enter_context(tc.tile_pool(name="xt", bufs=1))
        wqk_pool = top.enter_context(tc.tile_pool(name="wqk", bufs=2))
        wv_pool = top.enter_context(tc.tile_pool(name="wv", bufs=1))
        at_pool = top.enter_context(tc.tile_pool(name="at", bufs=1))
        wout_pool = top.enter_context(tc.tile_pool(name="wout", bufs=1))
        m01_pool = top.enter_context(tc.tile_pool(name="m01", bufs=1))
        pt_pool = top.enter_context(tc.tile_pool(name="pt", bufs=8))
        rs_pool = top.enter_context(tc.tile_pool(name="rs", bufs=2))
        bc_pool = top.enter_context(tc.tile_pool(name="bc", bufs=2))
        scr_pool = top.enter_context(
            tc.tile_pool(name="scr", bufs=2, space="DRAM"))
        stg_pool = top.enter_context(tc.tile_pool(name="stg", bufs=2))
        osb_pool = top.enter_context(tc.tile_pool(name="osb", bufs=4))
        ps_q = top.enter_context(tc.tile_pool(name="ps_q", bufs=2, space="PSUM"))
        ps_s = top.enter_context(tc.tile_pool(name="ps_s", bufs=2, space="PSUM"))
        ps_at = top.enter_context(tc.tile_pool(name="ps_at", bufs=1, space="PSUM"))
        ps_rs = top.enter_context(tc.tile_pool(name="ps_rs", bufs=1, space="PSUM"))

        ones_col = const.tile([P, 1], bf16, tag="ones_col")
        neg12 = const.tile([P, 1], f32, tag="neg12")
        bqk_sb = const.tile([P, 8], f32, tag="bqk")
        bv_bc = const.tile([P, 512], bf16, tag="bv_bc")
        nc.vector.memset(ones_col[:], 1.0)
        nc.vector.memset(neg12[:], -12.0)
        nc.sync.dma_start(bqk_sb[:], bqk_d[:])
        nc.sync.dma_start(bv_bc[:], bv_d[0:1, :].to_broadcast((P, 512)))
        m01_blk = m01_pool.tile([P, P], bf16, tag="m01")
        nc.sync.dma_start(m01_blk[:], m01_d[:])

        # qkT [1024, T] as 8 tiles (i<4: q of head pair i, pre-scaled 1/8;
        # i>=4: k of pair i-4); v [T, 512] as 16 tiles; attnT as 4 tiles.
        qk = [qk_pool.tile([P, T], bf16, tag=f"qk{i}", name=f"qk{i}")
              for i in range(8)]
        v = [v_pool.tile([P, 512], bf16, tag=f"v{t}", name=f"v{t}")
             for t in range(TTILES)]
        at = [at_pool.tile([P, T], bf16, tag=f"at{p}", name=f"at{p}")
              for p in range(4)]
        xt = [xt_pool.tile([P, T], bf16, tag=f"xt{kt}", name=f"xt{kt}")
              for kt in range(KT)]
        wv_sb = wv_pool.tile([P, KT * 512], bf16, tag="wv")
        wout_sb = wout_pool.tile([P, 4 * 1024], bf16, tag="wout")

        # weight DMAs for pair 0 first, then x chunk-major so the first
        # qk chunk can start after ~1.5 MB of input traffic.
        wqk_sb = {}
        for i in (0, 4):
            wqk_sb[i] = wqk_pool.tile([P, 1024], bf16, tag=f"wqk{i % 2}",
                                      name=f"wqk{i}")
            nc.sync.dma_start(wqk_sb[i][:], wqk_d[:, ts(i, 1024)])
        for kt in range(KT):
            nc.sync.dma_start(xt[kt][:, ts(0, CH)],
                              xt_d[:, kt * T + 0 * CH:kt * T + 1 * CH])
        nc.sync.dma_start(wv_sb[:], wv_d[:])
        for n in range(1, NCH):
            for kt in range(KT):
                nc.sync.dma_start(xt[kt][:, ts(n, CH)],
                                  xt_d[:, kt * T + n * CH:kt * T + (n + 1) * CH])
        nc.sync.dma_start(wout_sb[:], wout_d[:])

        def emit_qk_chunk(i, n):
            """qkT[i][:, chunk n] = (Wqk col-tile i).T @ x.T chunk"""
            acc = ps_q.tile([P, CH], f32, tag="q", name="qkacc")
            for kt in range(KT):
                nc.tensor.matmul(
                    acc[:], wqk_sb[i][:, ts(kt, P)], xt[kt][:, ts(n, CH)],
                    start=(kt == 0), stop=(kt == KT - 1))
            nc.vector.tensor_scalar_add(
                qk[i][:, ts(n, CH)], acc[:], bqk_sb[:, i:i + 1])

        def emit_v(t):
            """v[t] = x-tile.T @ Wv + bv          -> [128 tok, 512 dcol]"""
            acc = ps_q.tile([P, 512], f32, tag="q", name="vacc")
            for kt in range(KT):
                nc.tensor.matmul(
                    acc[:], xt[kt][:, ts(t, P)], wv_sb[:, ts(kt, 512)],
                    start=(kt == 0), stop=(kt == KT - 1))
            nc.vector.tensor_add(v[t][:], acc[:], bv_bc[:])

        def emit_op(c, t, dc):
            """out-projection chain for token tile t, d_model half dc"""
            acc = ps_q.tile([P, 512], f32, tag="q", name="oacc")
            for kk in range(4):
                nc.tensor.matmul(
                    acc[:], at[kk][:, ts(t, P)],
                    wout_sb[:, kk * 1024 + dc * 512:kk * 1024 + dc * 512 + 512],
                    start=(kk == 0), stop=(kk == 3))
            o_sb = osb_pool.tile([P, 512], f32, tag="o_sb")
            nc.vector.tensor_copy(o_sb[:], acc[:])
            nc.sync.dma_start(out_d[ts(t, P), ts(dc, 512)], o_sb[:])

        # ---- filler machinery -------------------------------------------
        # (kind, key, cost_ns, closure).  Fillers are emitted with natural
        # (low-preference) priority; attention blocks are emitted inside
        # tc.high_priority() so the scheduler runs them the moment they are
        # ready and uses fillers only for true PE gaps.
        fillers = deque()

        FILLER_DEMOTE = -10_000_000  # priority += 10M -> lowest preference

        def drain(pred):
            keep = deque()
            with tc.high_priority(offset=FILLER_DEMOTE):
                while fillers:
                    item = fillers.popleft()
                    if pred(item):
                        item[3]()
                    else:
                        keep.append(item)
            fillers.extend(keep)

        def att_block(c, p):
            # prerequisites: qk pair p fully emitted, v tiles 0..4c+3
            drain(lambda it: (it[0] == "qk" and it[1][0] == p)
                  or (it[0] == "v" and it[1] <= 4 * c + 3))
            _att_block_body(c, p)

        def _att_block_body(c, p):
            # Three priority classes: the scores->exp->av chain runs at
            # high priority (keeps tile-position pairs adjacent =>
            # concurrent in the array), rowsums at natural priority (they
            # only gate the block-end recip, not the exp pipeline), and
            # fillers demoted to lowest preference.
            kq = qk[4 + p]
            qq = qk[p]
            nki = 4 * (c + 1)
            attn_ps = ps_at.tile([P, CH], f32, tag="at")
            rs_ps = ps_rs.tile([P, CH], f32, tag="rs")
            for j in range(nki):
                st = (j == 0)
                sp = (j == nki - 1)
                # columns < off of this [ki, qi] tile are fully masked
                off = max(0, P * (j - 4 * c))
                pt = pt_pool.tile([P, 1024], bf16, tag="pt")
                with tc.high_priority():
                    s_ps = ps_s.tile([P, 1024], f32, tag="s", name="s_ps")
                    nc.tensor.matmul(
                        s_ps[:, off:CH], kq[0:DK, ts(j, P)],
                        qq[0:DK, c * CH + off:(c + 1) * CH],
                        start=True, stop=True)
                    nc.tensor.matmul(
                        s_ps[:, CH + off:1024], kq[DK:P, ts(j, P)],
                        qq[DK:P, c * CH + off:(c + 1) * CH],
                        start=True, stop=True)
                    if off > 0:
                        for base in (0, CH):
                            nc.scalar.activation(
                                pt[:, base + off:base + CH],
                                s_ps[:, base + off:base + CH],
                                EXP, bias=neg12[:], scale=1.0)
                    else:
                        nc.scalar.activation(
                            pt[:], s_ps[:], EXP, bias=neg12[:], scale=1.0)
                    if j >= 4 * c:  # tile containing the diagonal block
                        for base in (0, CH):
                            nc.vector.tensor_mul(
                                pt[:, base + off:base + off + P],
                                pt[:, base + off:base + off + P],
                                m01_blk[:])
                    nc.tensor.matmul(
                        attn_ps[0:DK, off:CH], v[j][:, ts(2 * p, DK)],
                        pt[:, off:CH],
                        start=st, stop=sp, skip_group_check=True)
                    nc.tensor.matmul(
                        attn_ps[DK:P, off:CH], v[j][:, ts(2 * p + 1, DK)],
                        pt[:, CH + off:1024],
                        start=st, stop=sp, skip_group_check=True)
                # rowsums gate only the block-end recip: natural priority
                nc.tensor.matmul(
                    rs_ps[0:1, off:CH], ones_col[:], pt[:, off:CH],
                    start=st, stop=sp, skip_group_check=True)
                nc.tensor.matmul(
                    rs_ps[32:33, off:CH], ones_col[:], pt[:, CH + off:1024],
                    start=st, stop=sp, skip_group_check=True)
            with tc.high_priority():
                # single recip over rows 0..32 (covers both heads' sums;
                # rows 1-31 are don't-care) — sub-partition DVE writes at
                # base>0 mis-track against downstream readers.  Frees the
                # rowsum bank immediately.
                rs_sb = rs_pool.tile([33, CH], f32, tag="rs_sb")
                nc.vector.reciprocal_approx_fast(rs_sb[:], rs_ps[0:33, :])
                # stage raw attention to SBUF now: frees the attn PSUM
                # bank in ~0.6us instead of waiting for the broadcast DMA.
                att_sb = stg_pool.tile([P, CH], bf16, tag="stg")
                nc.vector.tensor_copy(att_sb[:], attn_ps[:])
            # broadcast 1/rowsum across partitions via DRAM roundtrip DMA
            # and normalize off the critical path (gates only out-proj).
            scrA = scr_pool.tile([1, CH], f32, tag="scrA")
            scrB = scr_pool.tile([1, CH], f32, tag="scrB")
            nc.sync.dma_start(scrA[:], rs_sb[0:1, :])
            nc.sync.dma_start(scrB[:], rs_sb[32:33, :])
            bc_sb = bc_pool.tile([P, CH], f32, tag="bc")
            nc.sync.dma_start(bc_sb[0:DK, :],
                              scrA[0:1, :].to_broadcast((DK, CH)))
            nc.sync.dma_start(bc_sb[DK:P, :],
                              scrB[0:1, :].to_broadcast((DK, CH)))
            nc.vector.tensor_mul(
                at[p][:, ts(c, CH)], att_sb[:], bc_sb[:])

        # ---- prefix: pair 0 qkv, then pipelined attention ----------------
        for n in range(NCH):
            emit_qk_chunk(0, n)
            emit_qk_chunk(4, n)
        for t in range(4):
            emit_v(t)

        # queue the rest as fillers (qk keyed by (pair, chunk))
        for p in range(1, 4):
            def load_wqk(p=p):
                for i in (p, p + 4):
                    wqk_sb[i] = wqk_pool.tile(
                        [P, 1024], bf16, tag=f"wqk{i % 2}", name=f"wqk{i}")
                    nc.sync.dma_start(wqk_sb[i][:], wqk_d[:, ts(i, 1024)])
            fillers.append(("qk", (p, 0), 100.0, load_wqk))
            for n in range(NCH):
                fillers.append(("qk", (p, n), 1700.0,
                                lambda p=p, n=n: emit_qk_chunk(p, n)))
                fillers.append(("qk", (p, n), 1700.0,
                                lambda p=p, n=n: emit_qk_chunk(p + 4, n)))
        for t in range(4, TTILES):
            fillers.append(("v", t, 1700.0, lambda t=t: emit_v(t)))

        done_pairs = {c: 0 for c in range(NCH)}
        for c, p in BLOCK_ORDER:
            att_block(c, p)
            done_pairs[c] += 1
            if done_pairs[c] == 4:  # out-projection for chunk c unlocked
                for t in range(4 * c, 4 * c + 4):
                    for dc in range(2):
                        fillers.append(
                            ("op", c, 900.0,
                             lambda c=c, t=t, dc=dc: emit_op(c, t, dc)))
        # drain remaining fillers (tail out-projections)
        drain(lambda it: True)

    nc.compile()
    return nc


def _get_program():
    if "nc" not in _CACHE:
        _CACHE["nc"] = _build_program()
    return _CACHE["nc"]


def _prep_core_inputs(x, attn_mask, Wqkv, bqkv, Wout):
    """Per-core host-side sharding + DMA-friendly layouts."""
    # partial diagonal block: m01[ki_rel, qi_rel] = 1 iff qi_rel >= ki_rel
    m01 = np.triu(np.ones((P, P), np.float32)).astype(BF16)

    in_maps = []
    for core in range(NCORES):
        b, g = core // 2, core % 2
        xt = np.ascontiguousarray(
            x[b].T.reshape(KT, P, T).transpose(1, 0, 2).reshape(P, KT * T)
        ).astype(BF16)
        wq = Wqkv[:, 512 * g:512 * g + 512] * np.float32(0.125)
        wk = Wqkv[:, 1024 + 512 * g:1024 + 512 * g + 512]
        wqk = np.concatenate([wq, wk], axis=1)  # [1024, 1024]
        wqk = np.ascontiguousarray(
            wqk.reshape(KT, P, 8, P).transpose(1, 2, 0, 3).reshape(P, 8192)
        ).astype(BF16)
        wv = Wqkv[:, 2048 + 512 * g:2048 + 512 * g + 512]
        wv = np.ascontiguousarray(
            wv.reshape(KT, P, 512).transpose(1, 0, 2).reshape(P, KT * 512)
        ).astype(BF16)
        wo = Wout[512 * g:512 * g + 512, :]
        wo = np.ascontiguousarray(
            wo.reshape(4, P, 1024).transpose(1, 0, 2).reshape(P, 4096)
        ).astype(BF16)
        bq = bqkv[512 * g:512 * g + 512] * np.float32(0.125)
        bk = bqkv[1024 + 512 * g:1024 + 512 * g + 512]
        bqk = np.ascontiguousarray(
            np.concatenate([bq, bk]).reshape(8, P).T)
        bv = np.ascontiguousarray(
            bqkv[2048 + 512 * g:2048 + 512 * g + 512].reshape(1, 512)
        ).astype(BF16)
        in_maps.append({"xt": xt, "wqk": wqk, "wv": wv, "wout": wo,
                        "m01": m01, "bqk": bqk, "bv": bv})
    return in_maps


def _mask_is_causal(attn_mask):
    zero = (attn_mask == 0.0)
    if not np.array_equal(zero, np.tril(np.ones((T, T), dtype=bool))):
        return False
    return bool(np.all(attn_mask[~zero] <= np.float32(-50.0)))


def _numpy_fallback(x, attn_mask, Wqkv, bqkv, Wout, bout):
    qkv = x @ Wqkv + bqkv
    qkv = qkv.reshape(B, T, 3, H, DK).transpose(2, 0, 3, 1, 4)
    q, k, vv = qkv[0], qkv[1], qkv[2]
    scores = np.einsum("bhqd,bhkd->bhqk", q, k) / np.float32(np.sqrt(DK))
    scores = scores + attn_mask
    scores -= scores.max(axis=-1, keepdims=True)
    e = np.exp(scores)
    probs = e / e.sum(axis=-1, keepdims=True)
    attn = np.einsum("bhqk,bhkd->bhqd", probs, vv)
    attn = attn.transpose(0, 2, 1, 3).reshape(B, T, D)
    return (attn @ Wout + bout).astype(np.float32)


def _run(inputs, trace=False):
    from concourse.bass_utils import run_bass_kernel_spmd

    x = np.asarray(inputs["x"], dtype=np.float32)
    attn_mask = np.asarray(inputs["attn_mask"], dtype=np.float32)
    Wqkv = np.asarray(inputs["Wqkv"], dtype=np.float32)
    bqkv = np.asarray(inputs["bqkv"], dtype=np.float32)
    Wout = np.asarray(inputs["Wout"], dtype=np.float32)
    bout = np.asarray(inputs["bout"], dtype=np.float32)

    if not _mask_is_causal(attn_mask):
        return _numpy_fallback(x, attn_mask, Wqkv, bqkv, Wout, bout), None

    nc = _get_program()
    in_maps = _prep_core_inputs(x, attn_mask, Wqkv, bqkv, Wout)
    res = run_bass_kernel_spmd(nc, in_maps, list(range(NCORES)), trace=trace)
    out = np.empty((B, T, D), np.float32)
    for b in range(B):
        out[b] = res.results[2 * b]["out"] + res.results[2 * b + 1]["out"] + bout
    return out, res.exec_time_ns


def kernel(**inputs) -> np.ndarray:
    out, _ = _run(inputs, trace=False)
    return out
